# revision 34
# baseline (speedup 1.0000x reference)
"""Trainium2 Bass kernel for the supervised-contrastive loss (nn_KCL_69784628626020).

Strategy (8 NeuronCores, SPMD), v3:
  - Shard anchors (rows of q, k, y) across cores: 1024 rows/core.
  - Each core computes its [1024, 8192] slab of EW = exp((q_loc @ q_full^T
    + lw_j)/TAU) = w_j * exp(S/TAU) on the tensor engine (bf16 operands,
    fp32 PSUM, a rank-1 matmul folds the per-column lw_j = -TAU*ln(count_j)).
  - COLUMN ROTATION: core r's column order is rolled by r*1024 so that the
    self-similarity (diagonal) entry of local row-block b always lands in the
    static window [b*128, (b+1)*128).  A tiny eye-masked reduce extracts the
    exact stored w*E_ii per row.
  - Per row i (sums include the diagonal; it cancels exactly):
        A_i  = sum_j w_j E_ij       (FREE: activation accum_out during exp)
        SCw_i = sum_{y_j==y_i} w_j E_ij   (DVE fused masked reduce / quarter)
        den_i = log(A_i - SCw_i)          (diagonal cancels exactly)
        num_i = log(kpos_i + count_i*(SCw_i - wE_ii))
        loss_i = (den_i - num_i) / (count_i - 1 + K)
  - Class counts are computed on device (DVE label-equality reduces); the
    rotated lw row is assembled with a mask-shifted ReduceScatter (core r
    contributes lw_r into shard c slot (r-c)%8, so the scatter hands each
    core its rotated vector) -- pure data-driven, SPMD-safe.
  - kpos_i = sum_k exp(q_i . k_ik / TAU): GpSimd multiplies, ACT accumulates
    + exponentiates.
  - Final mean: ones-matmul partition reduction; host adds the 8 partials.
"""

import numpy as np
from contextlib import ExitStack

import concourse.bass as bass
import concourse.bacc as bacc
import concourse.tile as tile
from concourse import mybir
from concourse.bass_utils import run_bass_kernel_spmd
import ml_dtypes

F32 = mybir.dt.float32
F16 = mybir.dt.float16
BF16 = mybir.dt.bfloat16
AL = mybir.AluOpType
AF = mybir.ActivationFunctionType

TAU = 0.07
NCORES = 8


class Cfg:
    def __init__(self, N=8192, D=512, KP=8, NQ=4):
        self.N, self.D, self.KP, self.NQ = N, D, KP, NQ
        self.NL = N // NCORES      # rows per core
        self.NB = self.NL // 128   # 128-row blocks per core
        self.KC = D // 128         # contraction chunks
        self.QW = N // NQ          # column quarter width
        self.NCH = max(1, self.QW // 512)
        self.CW = self.QW // self.NCH   # matmul chunk width
        assert self.NL % 128 == 0 and self.QW % self.NCH == 0
        assert self.CW <= 512


def build_bass(cfg: Cfg, debug_out=False):
    N, D, KP, NQ = cfg.N, cfg.D, cfg.KP, cfg.NQ
    NL, NB, KC, QW, NCH, CW = cfg.NL, cfg.NB, cfg.KC, cfg.QW, cfg.NCH, cfg.CW
    NQT = NB * NQ              # total quarters

    nc = bacc.Bacc("TRN2", target_bir_lowering=False, debug=False,
                   num_devices=NCORES)

    # ---- kernel I/O -------------------------------------------------------
    qT_d = nc.dram_tensor("qT", [KC, 128, N], BF16, kind="ExternalInput")
    kr_d = nc.dram_tensor("kr", [NB, 128, KP * D], BF16, kind="ExternalInput")
    qr_d = nc.dram_tensor("qr", [NB, 128, D], BF16, kind="ExternalInput")
    ybc_d = nc.dram_tensor("ybc", [128, N], F16, kind="ExternalInput")
    yrow_d = nc.dram_tensor("yrow", [128, NB], F32, kind="ExternalInput")
    eyem_d = nc.dram_tensor("eyem", [128, 128], F16, kind="ExternalInput")
    m8_d = nc.dram_tensor("m8", [8, 8], BF16, kind="ExternalInput")
    zfeat_d = nc.dram_tensor("zfeat", [6, N], F16, kind="ExternalInput")
    zrow_d = nc.dram_tensor("zrow", [6, NL], F16, kind="ExternalInput")
    out_d = nc.dram_tensor("out", [1, 1], F32, kind="ExternalOutput")
    if debug_out:
        dlw_d = nc.dram_tensor("dlw", [1, N], BF16, kind="ExternalOutput")
        dfin_d = nc.dram_tensor("dfin", [128, 8 * NB], F32, kind="ExternalOutput")

    with tile.TileContext(nc) as tc, ExitStack() as ctx:
        const = ctx.enter_context(tc.tile_pool(name="const", bufs=1))
        ew_pool = ctx.enter_context(tc.tile_pool(name="ew", bufs=6))
        psum_pool = ctx.enter_context(tc.tile_pool(name="ps", bufs=2, space="PSUM"))
        k_pool = ctx.enter_context(tc.tile_pool(name="kp", bufs=1))
        q_pool = ctx.enter_context(tc.tile_pool(name="qp", bufs=2))
        ks_pool = ctx.enter_context(tc.tile_pool(name="ks", bufs=1))
        dram = ctx.enter_context(tc.tile_pool(name="dram", bufs=1, space="DRAM"))

        # ---- resident constants ------------------------------------------
        qts = [const.tile([128, N], BF16, tag=f"qts{c}", name=f"qts{c}")
               for c in range(KC)]
        ybc = const.tile([128, N], F16, tag="ybc")
        yrow = const.tile([128, NB], F32, tag="yrow")
        nc.sync.dma_start(yrow[:, :], yrow_d[:, :])
        eyem = const.tile([128, 128], F16, tag="eyem")
        nc.sync.dma_start(eyem[:, :], eyem_d[:, :])
        m8 = const.tile([8, 8], BF16, tag="m8")
        nc.sync.dma_start(m8[:, :], m8_d[:, :])
        zrow = const.tile([6, NL], F16, tag="zrow")
        nc.sync.dma_start(zrow[:, :], zrow_d[:, :])
        zfeat = const.tile([6, N], F16, tag="zfeat")
        nc.sync.dma_start(zfeat[:, :], zfeat_d[:, :])
        nc.sync.dma_start(ybc[:, 0:QW], ybc_d[:, 0:QW])
        nc.sync.dma_start(ybc[:, QW:N], ybc_d[:, QW:N])
        for c in range(KC):
            nc.sync.dma_start(qts[c][:, 0:QW], qT_d[c, :, 0:QW])
        # k-path inputs early so GpSimd products (and ACT reduces) start soon
        kts = []
        qts_k = []
        for b in range(NB):
            kt = k_pool.tile([128, KP * D], BF16, tag="kt", name=f"kt{b}")
            nc.sync.dma_start(kt[:, :], kr_d[b, :, :])
            qt = q_pool.tile([128, D], BF16, tag="qt", name=f"qt{b}")
            nc.sync.dma_start(qt[:, :], qr_d[b, :, :])
            kts.append(kt)
            qts_k.append(qt)
        for qq in range(1, NQ):
            for c in range(KC):
                nc.sync.dma_start(qts[c][:, qq * QW:(qq + 1) * QW],
                                  qT_d[c, :, qq * QW:(qq + 1) * QW])

        ones_col = const.tile([128, 1], F32, tag="ones_col")
        nc.vector.memset(ones_col[:, :], 1.0)
        ones_k1 = const.tile([1, 128], BF16, tag="ones_k1")
        nc.vector.memset(ones_k1[:, :], 1.0)

        # accumulator slots
        scs = const.tile([128, NQT], F32, tag="scs")
        aw = const.tile([128, NQT], F32, tag="aw")
        eh = const.tile([128, NB], F32, tag="eh")
        kss = const.tile([128, NB * KP], F32, tag="kss")
        kpos = const.tile([128, NB], F32, tag="kpos")
        cloc = const.tile([128, NB], F32, tag="cloc")
        clocB = const.tile([128, NB], F32, tag="clocB")

        # ---- k-path products on GpSimd (kt, qt ready early) --------------
        kscrs = []
        for b in range(NB):
            kscr = ks_pool.tile([128, KP * D], BF16, tag="kscr", name=f"kscr{b}")
            for kk in range(KP):
                nc.gpsimd.tensor_tensor(
                    kscr[:, kk * D:(kk + 1) * D],
                    kts[b][:, kk * D:(kk + 1) * D], qts_k[b][:, :], op=AL.mult)
            kscrs.append(kscr)

        # ---- class counts via PE z-trick + ACT exp-accum -----------------
        # z_ij = (hi_i-hi_j)^2 + (lo_i-lo_j)^2 is an exact small integer in
        # fp16/fp32 (labels split base-32); count_i = sum_j exp(-30*z_ij).
        # Runs on the otherwise-idle PE/ACT while lwrow is being assembled.
        zcs = const.tile([128, NQT], F32, tag="zcs")
        zdump = const.tile([128, QW], F32, tag="zdump")
        for b in range(NB):
            for qq in range(NQ):
                zps = psum_pool.tile([128, QW], F32, tag="ps")
                for ch in range(NCH):
                    nc.tensor.matmul(
                        zps[:, ch * CW:(ch + 1) * CW],
                        zrow[:, b * 128:(b + 1) * 128],
                        zfeat[:, qq * QW + ch * CW: qq * QW + (ch + 1) * CW],
                        start=True, stop=True)
                nc.scalar.activation(zdump[:, :], zps[:, :], AF.Exp,
                                     scale=float(-30.0),
                                     accum_out=zcs[:, b * NQ + qq:
                                                   b * NQ + qq + 1])
        nc.vector.tensor_reduce(
            cloc[:, :], zcs[:, :].rearrange("p (b q) -> p b q", b=NB, q=NQ),
            mybir.AxisListType.X, AL.add)

        # ---- lw row: lw = -TAU*ln(count); rotated share via RS -----------
        lnc = const.tile([128, NB], F32, tag="lnc")
        nc.scalar.activation(lnc[:, :], cloc[:, :], AF.Ln)
        lwloc = const.tile([128, NB], F32, tag="lwloc")
        nc.vector.tensor_scalar_mul(lwloc[:, :], lnc[:, :], -TAU)
        lw16 = const.tile([128, NB], BF16, tag="lw16")
        nc.vector.tensor_copy(lw16[:, :], lwloc[:, :])
        lwpad = const.tile([128, 32], BF16, tag="lwpad")
        nc.vector.memset(lwpad[:, :], 0.0)
        nc.vector.tensor_copy(lwpad[:, 0:NB], lw16[:, :])
        lwT = const.tile([128, 32], BF16, tag="lwT")
        nc.vector.transpose(lwT[:, :], lwpad[:, :])
        # lwpart[0, i*128+a*32+j] = lw(local row i*128+a*32+j) = lwT[a*32+i, j]
        lwpart = dram.tile([1, NL], BF16)
        dst3 = lwpart[:, :].rearrange("o (i a j) -> a i (o j)", i=NB, a=4, j=32)
        for a in range(4):
            nc.gpsimd.dma_start(dst3[a], lwT[a * 32:a * 32 + NB, 0:32])
        # tiny AllGather (2KB/core), then rotate ON-core with a marshalled
        # 8x8 permutation matmul: lwrot[s, g] = wall[(r+s)%8, g]
        wall_d = dram.tile([NCORES, NL], BF16, addr_space="Shared")
        nc.gpsimd.collective_compute(
            "AllGather", AL.bypass,
            ins=[lwpart[:, :].opt()],
            outs=[wall_d[:, :].opt()],
            replica_groups=[list(range(NCORES))],
        )
        wall_sb = const.tile([8, NL], BF16, tag="wall_sb")
        nc.gpsimd.dma_start(wall_sb[:, :], wall_d[:, :])
        lps = psum_pool.tile([128, QW], F32, tag="ps")
        for hh in range(max(1, NL // 512)):
            cw8 = min(512, NL)
            nc.tensor.matmul(lps[0:8, hh * cw8:(hh + 1) * cw8],
                             m8[:, :], wall_sb[:, hh * cw8:(hh + 1) * cw8],
                             start=True, stop=True)
        l8 = const.tile([8, NL], BF16, tag="l8")
        nc.scalar.copy(l8[:, :], lps[0:8, 0:NL])
        lwrow = const.tile([1, N], BF16, tag="lwrow")
        nc.gpsimd.dma_start(
            lwrow[:, :].rearrange("o (s g) -> o s g", s=NCORES, g=NL),
            l8[:, :])

        # ---- main loop ----------------------------------------------------
        # PE: quarters of the weighted score slab (q-chunks + lw rank-1).
        # ACT: exp with accum (A row-sum free) + k-path reduces interleaved.
        # DVE: SCw masked reduce + diag extract.
        sc_scr = const.tile([128, QW], F32, tag="sc_scr")
        eh_scr = const.tile([128, 128], F32, tag="eh_scr")
        kdump = const.tile([128, D], BF16, tag="kdump")

        kred_jobs = [(b, kk) for b in range(NB) for kk in range(KP)]
        kred_pos = 0
        kred_per_q = (len(kred_jobs) + NQT - 2) // max(1, NQT - 1)

        def emit_kred(n):
            nonlocal kred_pos
            for _ in range(n):
                if kred_pos >= len(kred_jobs):
                    return
                b, kk = kred_jobs[kred_pos]
                nc.scalar.activation(
                    kdump[:, :],
                    kscrs[b][:, kk * D:(kk + 1) * D],
                    AF.Copy, accum_out=kss[:, b * KP + kk: b * KP + kk + 1])
                kred_pos += 1

        for b in range(NB):
            q0t = None
            for qq in range(NQ):
                m = b * NQ + qq
                ewt = ew_pool.tile([128, QW], F32, tag="ew", name=f"ew{m}")
                if qq == 0:
                    q0t = ewt
                ps = psum_pool.tile([128, QW], F32, tag="ps")
                for c in range(KC):
                    for ch in range(NCH):
                        o = ps[:, ch * CW:(ch + 1) * CW]
                        nc.tensor.matmul(
                            o,
                            qts[c][:, b * 128:(b + 1) * 128],
                            qts[c][:, qq * QW + ch * CW: qq * QW + (ch + 1) * CW],
                            start=(c == 0), stop=False)
                for ch in range(NCH):
                    o = ps[:, ch * CW:(ch + 1) * CW]
                    nc.tensor.matmul(
                        o, ones_k1[0:1, :],
                        lwrow[0:1, qq * QW + ch * CW: qq * QW + (ch + 1) * CW],
                        start=False, stop=True)
                ewq = ewt[:, :]
                nc.scalar.activation(ewq, ps[:, :], AF.Exp,
                                     scale=float(1.0 / TAU),
                                     accum_out=aw[:, m:m + 1])
                if m >= 1:
                    emit_kred(kred_per_q)
                # SCw: same-class weighted row-sum (incl diag) on DVE
                nc.vector.scalar_tensor_tensor(
                    sc_scr[:, :], ybc[:, qq * QW:(qq + 1) * QW],
                    yrow[:, b:b + 1], ewq,
                    op0=AL.is_equal, op1=AL.mult,
                    accum_out=scs[:, m:m + 1])
            # exact diagonal extraction from the static rotated window
            # (local block b's diagonal lies in quarter 0 of its slab)
            nc.vector.scalar_tensor_tensor(
                eh_scr[:, :], eyem[:, :], 1.0, q0t[:, b * 128:(b + 1) * 128],
                op0=AL.mult, op1=AL.mult,
                accum_out=eh[:, b:b + 1])

        emit_kred(len(kred_jobs))

        # ---- k-path exps --------------------------------------------------
        ksse = const.tile([128, NB * KP], F32, tag="ksse")
        for b in range(NB):
            nc.scalar.activation(
                ksse[:, b * KP:(b + 1) * KP],
                kss[:, b * KP:(b + 1) * KP],
                AF.Exp, scale=float(1.0 / TAU),
                accum_out=kpos[:, b:b + 1])

        # ---- finalize (wide [128, NB] ops) --------------------------------
        SC = const.tile([128, NB], F32, tag="SC")
        A = const.tile([128, NB], F32, tag="A")
        nc.vector.tensor_reduce(
            SC[:, :], scs[:, :].rearrange("p (b q) -> p b q", b=NB, q=NQ),
            mybir.AxisListType.X, AL.add)
        nc.vector.tensor_reduce(
            A[:, :], aw[:, :].rearrange("p (b q) -> p b q", b=NB, q=NQ),
            mybir.AxisListType.X, AL.add)
        numin = const.tile([128, NB], F32, tag="numin")
        tmp = const.tile([128, NB], F32, tag="tmp")
        densub = const.tile([128, NB], F32, tag="densub")
        # num_in = kpos + cloc * (SCw - eh)
        nc.vector.tensor_sub(tmp[:, :], SC[:, :], eh[:, :])
        nc.vector.tensor_mul(tmp[:, :], tmp[:, :], cloc[:, :])
        nc.vector.tensor_add(numin[:, :], tmp[:, :], kpos[:, :])
        # den_in = A - SCw  (diagonal and same-class weights cancel exactly)
        nc.vector.tensor_sub(densub[:, :], A[:, :], SC[:, :])
        den_l = const.tile([128, NB], F32, tag="den_l")
        num_l = const.tile([128, NB], F32, tag="num_l")
        nc.scalar.activation(den_l[:, :], densub[:, :], AF.Ln)
        nc.scalar.activation(num_l[:, :], numin[:, :], AF.Ln)
        # loss rows: (den_l - num_l) / (cloc - 1 + KP)
        ctil = const.tile([128, NB], F32, tag="ctil")
        dinv = const.tile([128, NB], F32, tag="dinv")
        nc.vector.tensor_scalar_add(ctil[:, :], cloc[:, :], float(KP - 1))
        nc.vector.reciprocal(dinv[:, :], ctil[:, :])
        diff = const.tile([128, NB], F32, tag="diff")
        lossrow = const.tile([128, NB], F32, tag="lossrow")
        nc.vector.tensor_sub(diff[:, :], den_l[:, :], num_l[:, :])
        nc.vector.tensor_mul(lossrow[:, :], diff[:, :], dinv[:, :])

        # ---- reduce to a single partial ----------------------------------
        lsum = const.tile([128, 1], F32, tag="lsum")
        nc.vector.tensor_reduce(lsum[:, :], lossrow[:, :],
                                mybir.AxisListType.X, AL.add)
        psf = psum_pool.tile([128, QW], F32, tag="ps")
        nc.tensor.matmul(psf[0:1, 0:1], lsum[:, :],
                         ones_col[:, :], start=True, stop=True)
        outsb = const.tile([1, 1], F32, tag="outsb")
        nc.scalar.copy(outsb[0:1, 0:1], psf[0:1, 0:1])
        nc.sync.dma_start(out_d[:, :], outsb[0:1, 0:1])

        if debug_out:
            nc.sync.dma_start(dlw_d[:, :], lwrow[0:1, :])
            dfin = const.tile([128, 8 * NB], F32, tag="dfin")
            for i, t in enumerate([SC, A, eh, kpos, cloc, densub, numin,
                                   lossrow]):
                nc.vector.tensor_copy(dfin[:, i * NB:(i + 1) * NB], t[:, :])
            nc.sync.dma_start(dfin_d[:, :], dfin[:, :])

    nc.compile()
    return nc


# ---------------------------------------------------------------------------
# host-side marshalling
# ---------------------------------------------------------------------------

def make_inputs(q, k, y, cfg: Cfg):
    """Build the per-core input maps (pure layout/replication marshalling)."""
    N, D, KP = cfg.N, cfg.D, cfg.KP
    NL, NB, KC = cfg.NL, cfg.NB, cfg.KC
    q = np.asarray(q, dtype=np.float32)
    k = np.asarray(k, dtype=np.float32)
    y = np.asarray(y)

    qbf = q.astype(ml_dtypes.bfloat16)
    qTf = np.ascontiguousarray(qbf.T)           # [D, N]
    eyem = np.eye(128, dtype=np.float16)

    in_maps = []
    for r in range(NCORES):
        rows = slice(r * NL, (r + 1) * NL)
        roll = (np.arange(N) + r * NL) % N
        qT = np.ascontiguousarray(qTf[:, roll]).reshape(KC, 128, N)
        ybc = np.broadcast_to(y[roll].astype(np.float16)[None, :], (128, N)).copy()
        yrow = np.ascontiguousarray(
            y[rows].astype(np.float32).reshape(NB, 128).T)
        kr = np.ascontiguousarray(
            k[rows].reshape(NB, 128, KP * D)).astype(ml_dtypes.bfloat16)
        qr = np.ascontiguousarray(qbf[rows].reshape(NB, 128, D))
        yh = (y // 32).astype(np.float16)
        yl = (y % 32).astype(np.float16)
        yhr, ylr = yh[roll], yl[roll]
        zfeat = np.stack([np.ones(N, np.float16), -2 * yhr, yhr * yhr,
                          np.ones(N, np.float16), -2 * ylr, ylr * ylr]).astype(np.float16)
        yhl, yll = yh[rows], yl[rows]
        zrow = np.stack([yhl * yhl, yhl, np.ones(NL, np.float16),
                         yll * yll, yll, np.ones(NL, np.float16)]).astype(np.float16)
        m8 = np.zeros((NCORES, NCORES), dtype=ml_dtypes.bfloat16)
        for s in range(NCORES):
            m8[(r + s) % NCORES, s] = 1.0
        in_maps.append({
            "qT": qT, "kr": kr, "qr": qr, "ybc": ybc, "yrow": yrow,
            "eyem": eyem, "m8": m8, "zfeat": zfeat, "zrow": zrow,
        })
    return in_maps


_CACHE = {}


def _get_nc(cfg_key):
    if cfg_key not in _CACHE:
        cfg = Cfg()
        _CACHE[cfg_key] = (cfg, build_bass(cfg))
    return _CACHE[cfg_key]


def kernel(q, k, y, trace=False):
    cfg, nc = _get_nc("full")
    in_maps = make_inputs(q, k, y, cfg)
    res = run_bass_kernel_spmd(nc, in_maps, core_ids=list(range(NCORES)),
                               trace=trace)
    total = np.sum([res.results[r]["out"][0, 0] for r in range(NCORES)],
                   dtype=np.float64)
    out = np.asarray(total / cfg.N, dtype=np.float32)
    if trace:
        kernel.last_results = res
    return out


# revision 36
# speedup vs baseline: 1.4680x; 1.4680x over previous
"""Trainium2 Bass kernel for the supervised-contrastive loss (nn_KCL_69784628626020).

Strategy (8 NeuronCores, SPMD), v3:
  - Shard anchors (rows of q, k, y) across cores: 1024 rows/core.
  - Each core computes its [1024, 8192] slab of EW = exp((q_loc @ q_full^T
    + lw_j)/TAU) = w_j * exp(S/TAU) on the tensor engine (bf16 operands,
    fp32 PSUM, a rank-1 matmul folds the per-column lw_j = -TAU*ln(count_j)).
  - COLUMN ROTATION: core r's column order is rolled by r*1024 so that the
    self-similarity (diagonal) entry of local row-block b always lands in the
    static window [b*128, (b+1)*128).  A tiny eye-masked reduce extracts the
    exact stored w*E_ii per row.
  - Per row i (sums include the diagonal; it cancels exactly):
        A_i  = sum_j w_j E_ij       (FREE: activation accum_out during exp)
        SCw_i = sum_{y_j==y_i} w_j E_ij   (DVE fused masked reduce / quarter)
        den_i = log(A_i - SCw_i)          (diagonal cancels exactly)
        num_i = log(kpos_i + count_i*(SCw_i - wE_ii))
        loss_i = (den_i - num_i) / (count_i - 1 + K)
  - Class counts are computed on device (DVE label-equality reduces); the
    rotated lw row is assembled with a mask-shifted ReduceScatter (core r
    contributes lw_r into shard c slot (r-c)%8, so the scatter hands each
    core its rotated vector) -- pure data-driven, SPMD-safe.
  - kpos_i = sum_k exp(q_i . k_ik / TAU): GpSimd multiplies, ACT accumulates
    + exponentiates.
  - Final mean: ones-matmul partition reduction; host adds the 8 partials.
"""

import numpy as np
from contextlib import ExitStack

import concourse.bass as bass
import concourse.bacc as bacc
import concourse.tile as tile
from concourse import mybir
from concourse.bass_utils import run_bass_kernel_spmd
import ml_dtypes

F32 = mybir.dt.float32
F16 = mybir.dt.float16
BF16 = mybir.dt.bfloat16
AL = mybir.AluOpType
AF = mybir.ActivationFunctionType

TAU = 0.07
NCORES = 8


class Cfg:
    def __init__(self, N=8192, D=512, KP=8, NQ=4):
        self.N, self.D, self.KP, self.NQ = N, D, KP, NQ
        self.NL = N // NCORES      # rows per core
        self.NB = self.NL // 128   # 128-row blocks per core
        self.KC = D // 128         # contraction chunks
        self.QW = N // NQ          # column quarter width
        self.NCH = max(1, self.QW // 512)
        self.CW = self.QW // self.NCH   # matmul chunk width
        assert self.NL % 128 == 0 and self.QW % self.NCH == 0
        assert self.CW <= 512


def build_bass(cfg: Cfg, debug_out=False):
    N, D, KP, NQ = cfg.N, cfg.D, cfg.KP, cfg.NQ
    NL, NB, KC, QW, NCH, CW = cfg.NL, cfg.NB, cfg.KC, cfg.QW, cfg.NCH, cfg.CW
    NQT = NB * NQ              # total quarters

    nc = bacc.Bacc("TRN2", target_bir_lowering=False, debug=False,
                   num_devices=NCORES)

    # ---- kernel I/O -------------------------------------------------------
    qT_d = nc.dram_tensor("qT", [KC, 128, N], BF16, kind="ExternalInput")
    kr_d = nc.dram_tensor("kr", [NB, 128, KP * D], BF16, kind="ExternalInput")
    qr_d = nc.dram_tensor("qr", [NB, 128, D], BF16, kind="ExternalInput")
    ybc_d = nc.dram_tensor("ybc", [128, N], F16, kind="ExternalInput")
    yrow_d = nc.dram_tensor("yrow", [128, NB], F32, kind="ExternalInput")
    eyem_d = nc.dram_tensor("eyem", [128, 128], F16, kind="ExternalInput")
    mask8_d = nc.dram_tensor("mask8", [128, 1], F32, kind="ExternalInput")
    zfeat_d = nc.dram_tensor("zfeat", [6, N], F16, kind="ExternalInput")
    zrow_d = nc.dram_tensor("zrow", [6, NL], F16, kind="ExternalInput")
    out_d = nc.dram_tensor("out", [1, 1], F32, kind="ExternalOutput")
    if debug_out:
        dlw_d = nc.dram_tensor("dlw", [1, N], BF16, kind="ExternalOutput")
        dfin_d = nc.dram_tensor("dfin", [128, 8 * NB], F32, kind="ExternalOutput")

    with tile.TileContext(nc) as tc, ExitStack() as ctx:
        const = ctx.enter_context(tc.tile_pool(name="const", bufs=1))
        ew_pool = ctx.enter_context(tc.tile_pool(name="ew", bufs=6))
        psum_pool = ctx.enter_context(tc.tile_pool(name="ps", bufs=2, space="PSUM"))
        k_pool = ctx.enter_context(tc.tile_pool(name="kp", bufs=1))
        q_pool = ctx.enter_context(tc.tile_pool(name="qp", bufs=2))
        ks_pool = ctx.enter_context(tc.tile_pool(name="ks", bufs=1))
        dram = ctx.enter_context(tc.tile_pool(name="dram", bufs=1, space="DRAM"))

        # ---- resident constants ------------------------------------------
        qts = [const.tile([128, N], BF16, tag=f"qts{c}", name=f"qts{c}")
               for c in range(KC)]
        ybc = const.tile([128, N], F16, tag="ybc")
        yrow = const.tile([128, NB], F32, tag="yrow")
        nc.sync.dma_start(yrow[:, :], yrow_d[:, :])
        eyem = const.tile([128, 128], F16, tag="eyem")
        nc.sync.dma_start(eyem[:, :], eyem_d[:, :])
        mask8 = const.tile([128, 1], F32, tag="mask8")
        nc.sync.dma_start(mask8[:, :], mask8_d[:, :])
        zrow = const.tile([6, NL], F16, tag="zrow")
        nc.sync.dma_start(zrow[:, :], zrow_d[:, :])
        zfeat = const.tile([6, N], F16, tag="zfeat")
        nc.sync.dma_start(zfeat[:, :], zfeat_d[:, :])
        nc.sync.dma_start(ybc[:, 0:QW], ybc_d[:, 0:QW])
        nc.sync.dma_start(ybc[:, QW:N], ybc_d[:, QW:N])
        for c in range(KC):
            nc.sync.dma_start(qts[c][:, 0:QW], qT_d[c, :, 0:QW])
        # k-path inputs early so GpSimd products (and ACT reduces) start soon
        kts = []
        qts_k = []
        for b in range(NB):
            kt = k_pool.tile([128, KP * D], BF16, tag="kt", name=f"kt{b}")
            nc.sync.dma_start(kt[:, :], kr_d[b, :, :])
            qt = q_pool.tile([128, D], BF16, tag="qt", name=f"qt{b}")
            nc.sync.dma_start(qt[:, :], qr_d[b, :, :])
            kts.append(kt)
            qts_k.append(qt)
        for qq in range(1, NQ):
            for c in range(KC):
                nc.sync.dma_start(qts[c][:, qq * QW:(qq + 1) * QW],
                                  qT_d[c, :, qq * QW:(qq + 1) * QW])

        ones_col = const.tile([128, 1], F32, tag="ones_col")
        nc.vector.memset(ones_col[:, :], 1.0)
        ones_k1 = const.tile([1, 128], BF16, tag="ones_k1")
        nc.vector.memset(ones_k1[:, :], 1.0)

        # accumulator slots
        scs = const.tile([128, NQT], F32, tag="scs")
        aw = const.tile([128, NQT], F32, tag="aw")
        eh = const.tile([128, NB], F32, tag="eh")
        kss = const.tile([128, NB * KP], F32, tag="kss")
        kpos = const.tile([128, NB], F32, tag="kpos")
        cloc = const.tile([128, NB], F32, tag="cloc")
        clocB = const.tile([128, NB], F32, tag="clocB")

        # ---- k-path products on GpSimd (kt, qt ready early) --------------
        kscrs = []
        for b in range(NB):
            kscr = ks_pool.tile([128, KP * D], BF16, tag="kscr", name=f"kscr{b}")
            for kk in range(KP):
                nc.gpsimd.tensor_tensor(
                    kscr[:, kk * D:(kk + 1) * D],
                    kts[b][:, kk * D:(kk + 1) * D], qts_k[b][:, :], op=AL.mult)
            kscrs.append(kscr)

        # ---- class counts via PE z-trick + ACT exp-accum -----------------
        # z_ij = (hi_i-hi_j)^2 + (lo_i-lo_j)^2 is an exact small integer in
        # fp16/fp32 (labels split base-32); count_i = sum_j exp(-30*z_ij).
        # Runs on the otherwise-idle PE/ACT while lwrow is being assembled.
        zcs = const.tile([128, NQT], F32, tag="zcs")
        zdump = const.tile([128, QW], F32, tag="zdump")
        zdumpv = const.tile([128, QW], F16, tag="zdumpv")
        for b in range(NB):
            for qq in range(NQ):
                m = b * NQ + qq
                zps = psum_pool.tile([128, QW], F32, tag="ps")
                for ch in range(NCH):
                    nc.tensor.matmul(
                        zps[:, ch * CW:(ch + 1) * CW],
                        zrow[:, b * 128:(b + 1) * 128],
                        zfeat[:, qq * QW + ch * CW: qq * QW + (ch + 1) * CW],
                        start=True, stop=True)
                if m % 2 == 0:
                    nc.scalar.activation(zdump[:, :], zps[:, :], AF.Exp,
                                         scale=float(-30.0),
                                         accum_out=zcs[:, m:m + 1])
                else:
                    nc.vector.tensor_scalar(
                        zdumpv[:, :], zps[:, :], 0.0, None,
                        op0=AL.is_equal, op1=AL.add,
                        accum_out=zcs[:, m:m + 1])
        nc.vector.tensor_reduce(
            cloc[:, :], zcs[:, :].rearrange("p (b q) -> p b q", b=NB, q=NQ),
            mybir.AxisListType.X, AL.add)

        # ---- lw row: lw = -TAU*ln(count); rotated share via RS -----------
        lnc = const.tile([128, NB], F32, tag="lnc")
        nc.scalar.activation(lnc[:, :], cloc[:, :], AF.Ln)
        lwloc = const.tile([128, NB], F32, tag="lwloc")
        nc.vector.tensor_scalar_mul(lwloc[:, :], lnc[:, :], -TAU)
        lw16 = const.tile([128, NB], BF16, tag="lw16")
        nc.vector.tensor_copy(lw16[:, :], lwloc[:, :])
        lwpad = const.tile([128, 32], BF16, tag="lwpad")
        nc.vector.memset(lwpad[:, :], 0.0)
        nc.vector.tensor_copy(lwpad[:, 0:NB], lw16[:, :])
        lwT = const.tile([128, 32], BF16, tag="lwT")
        nc.vector.transpose(lwT[:, :], lwpad[:, :])
        # lwpart[0, i*128+a*32+j] = lw(local row i*128+a*32+j) = lwT[a*32+i, j]
        lwpart = dram.tile([1, NL], BF16)
        dst3 = lwpart[:, :].rearrange("o (i a j) -> a i (o j)", i=NB, a=4, j=32)
        for a in range(4):
            nc.gpsimd.dma_start(dst3[a], lwT[a * 32:a * 32 + NB, 0:32])
        # ReduceScatter rotation: core r contributes lw_r to shard c slot
        # (r-c)%8; the scatter hands core c exactly its rotated lw vector.
        bufc = const.tile([128, NL], BF16, tag="bufc")
        nc.gpsimd.dma_start(bufc[0:64, :],
                            lwpart[0:1, :].partition_broadcast(64))
        nc.vector.tensor_scalar_mul(bufc[0:64, :], bufc[0:64, :],
                                    mask8[0:64, 0:1])
        bufc_d = dram.tile([1, 64 * NL], BF16)
        nc.gpsimd.dma_start(
            bufc_d[:, :].rearrange("o (p g) -> p (o g)", p=64, g=NL),
            bufc[0:64, :])
        lwrs_d = dram.tile([1, NCORES * NL], BF16)
        nc.gpsimd.collective_compute(
            "ReduceScatter", AL.add,
            ins=[bufc_d[:, :].opt()],
            outs=[lwrs_d[:, :].opt()],
            replica_groups=[list(range(NCORES))],
        )
        lwrow = const.tile([1, N], BF16, tag="lwrow")
        nc.gpsimd.dma_start(lwrow[0:1, :], lwrs_d[0:1, :])

        # ---- main loop ----------------------------------------------------
        # PE: quarters of the weighted score slab (q-chunks + lw rank-1).
        # ACT: exp with accum (A row-sum free) + k-path reduces interleaved.
        # DVE: SCw masked reduce + diag extract.
        sc_scr = const.tile([128, QW], F32, tag="sc_scr")
        eh_scr = const.tile([128, 128], F32, tag="eh_scr")
        kdump = const.tile([128, D], BF16, tag="kdump")

        kred_jobs = [(b, kk) for b in range(NB) for kk in range(KP)]
        kred_pos = 0

        def emit_kred(n):
            nonlocal kred_pos
            for _ in range(n):
                if kred_pos >= len(kred_jobs):
                    return
                b, kk = kred_jobs[kred_pos]
                nc.scalar.activation(
                    kdump[:, :],
                    kscrs[b][:, kk * D:(kk + 1) * D],
                    AF.Copy, accum_out=kss[:, b * KP + kk: b * KP + kk + 1])
                kred_pos += 1

        emit_kred(len(kred_jobs))

        for b in range(NB):
            q0t = None
            for qq in range(NQ):
                m = b * NQ + qq
                ewt = ew_pool.tile([128, QW], F32, tag="ew", name=f"ew{m}")
                if qq == 0:
                    q0t = ewt
                ps = psum_pool.tile([128, QW], F32, tag="ps")
                for c in range(KC):
                    for ch in range(NCH):
                        o = ps[:, ch * CW:(ch + 1) * CW]
                        nc.tensor.matmul(
                            o,
                            qts[c][:, b * 128:(b + 1) * 128],
                            qts[c][:, qq * QW + ch * CW: qq * QW + (ch + 1) * CW],
                            start=(c == 0), stop=False)
                for ch in range(NCH):
                    o = ps[:, ch * CW:(ch + 1) * CW]
                    nc.tensor.matmul(
                        o, ones_k1[0:1, :],
                        lwrow[0:1, qq * QW + ch * CW: qq * QW + (ch + 1) * CW],
                        start=False, stop=True)
                ewq = ewt[:, :]
                nc.scalar.activation(ewq, ps[:, :], AF.Exp,
                                     scale=float(1.0 / TAU),
                                     accum_out=aw[:, m:m + 1])
                # SCw: same-class weighted row-sum (incl diag) on DVE
                nc.vector.scalar_tensor_tensor(
                    sc_scr[:, :], ybc[:, qq * QW:(qq + 1) * QW],
                    yrow[:, b:b + 1], ewq,
                    op0=AL.is_equal, op1=AL.mult,
                    accum_out=scs[:, m:m + 1])
            # exact diagonal extraction from the static rotated window
            # (local block b's diagonal lies in quarter 0 of its slab)
            nc.vector.scalar_tensor_tensor(
                eh_scr[:, :], eyem[:, :], 1.0, q0t[:, b * 128:(b + 1) * 128],
                op0=AL.mult, op1=AL.mult,
                accum_out=eh[:, b:b + 1])

        emit_kred(len(kred_jobs))

        # ---- k-path exps --------------------------------------------------
        ksse = const.tile([128, NB * KP], F32, tag="ksse")
        for b in range(NB):
            nc.scalar.activation(
                ksse[:, b * KP:(b + 1) * KP],
                kss[:, b * KP:(b + 1) * KP],
                AF.Exp, scale=float(1.0 / TAU),
                accum_out=kpos[:, b:b + 1])

        # ---- finalize (wide [128, NB] ops) --------------------------------
        SC = const.tile([128, NB], F32, tag="SC")
        A = const.tile([128, NB], F32, tag="A")
        nc.vector.tensor_reduce(
            SC[:, :], scs[:, :].rearrange("p (b q) -> p b q", b=NB, q=NQ),
            mybir.AxisListType.X, AL.add)
        nc.vector.tensor_reduce(
            A[:, :], aw[:, :].rearrange("p (b q) -> p b q", b=NB, q=NQ),
            mybir.AxisListType.X, AL.add)
        numin = const.tile([128, NB], F32, tag="numin")
        tmp = const.tile([128, NB], F32, tag="tmp")
        densub = const.tile([128, NB], F32, tag="densub")
        # num_in = kpos + cloc * (SCw - eh)
        nc.vector.tensor_sub(tmp[:, :], SC[:, :], eh[:, :])
        nc.vector.tensor_mul(tmp[:, :], tmp[:, :], cloc[:, :])
        nc.vector.tensor_add(numin[:, :], tmp[:, :], kpos[:, :])
        # den_in = A - SCw  (diagonal and same-class weights cancel exactly)
        nc.vector.tensor_sub(densub[:, :], A[:, :], SC[:, :])
        den_l = const.tile([128, NB], F32, tag="den_l")
        num_l = const.tile([128, NB], F32, tag="num_l")
        nc.scalar.activation(den_l[:, :], densub[:, :], AF.Ln)
        nc.scalar.activation(num_l[:, :], numin[:, :], AF.Ln)
        # loss rows: (den_l - num_l) / (cloc - 1 + KP)
        ctil = const.tile([128, NB], F32, tag="ctil")
        dinv = const.tile([128, NB], F32, tag="dinv")
        nc.vector.tensor_scalar_add(ctil[:, :], cloc[:, :], float(KP - 1))
        nc.vector.reciprocal(dinv[:, :], ctil[:, :])
        diff = const.tile([128, NB], F32, tag="diff")
        lossrow = const.tile([128, NB], F32, tag="lossrow")
        nc.vector.tensor_sub(diff[:, :], den_l[:, :], num_l[:, :])
        nc.vector.tensor_mul(lossrow[:, :], diff[:, :], dinv[:, :])

        # ---- reduce to a single partial ----------------------------------
        lsum = const.tile([128, 1], F32, tag="lsum")
        nc.vector.tensor_reduce(lsum[:, :], lossrow[:, :],
                                mybir.AxisListType.X, AL.add)
        psf = psum_pool.tile([128, QW], F32, tag="ps")
        nc.tensor.matmul(psf[0:1, 0:1], lsum[:, :],
                         ones_col[:, :], start=True, stop=True)
        outsb = const.tile([1, 1], F32, tag="outsb")
        nc.scalar.copy(outsb[0:1, 0:1], psf[0:1, 0:1])
        nc.sync.dma_start(out_d[:, :], outsb[0:1, 0:1])

        if debug_out:
            nc.sync.dma_start(dlw_d[:, :], lwrow[0:1, :])
            dfin = const.tile([128, 8 * NB], F32, tag="dfin")
            for i, t in enumerate([SC, A, eh, kpos, cloc, densub, numin,
                                   lossrow]):
                nc.vector.tensor_copy(dfin[:, i * NB:(i + 1) * NB], t[:, :])
            nc.sync.dma_start(dfin_d[:, :], dfin[:, :])

    nc.compile()
    return nc


# ---------------------------------------------------------------------------
# host-side marshalling
# ---------------------------------------------------------------------------

def make_inputs(q, k, y, cfg: Cfg):
    """Build the per-core input maps (pure layout/replication marshalling)."""
    N, D, KP = cfg.N, cfg.D, cfg.KP
    NL, NB, KC = cfg.NL, cfg.NB, cfg.KC
    q = np.asarray(q, dtype=np.float32)
    k = np.asarray(k, dtype=np.float32)
    y = np.asarray(y)

    qbf = q.astype(ml_dtypes.bfloat16)
    qTf = np.ascontiguousarray(qbf.T)           # [D, N]
    eyem = np.eye(128, dtype=np.float16)

    in_maps = []
    for r in range(NCORES):
        rows = slice(r * NL, (r + 1) * NL)
        roll = (np.arange(N) + r * NL) % N
        qT = np.ascontiguousarray(qTf[:, roll]).reshape(KC, 128, N)
        ybc = np.broadcast_to(y[roll].astype(np.float16)[None, :], (128, N)).copy()
        yrow = np.ascontiguousarray(
            y[rows].astype(np.float32).reshape(NB, 128).T)
        kr = np.ascontiguousarray(
            k[rows].reshape(NB, 128, KP * D)).astype(ml_dtypes.bfloat16)
        qr = np.ascontiguousarray(qbf[rows].reshape(NB, 128, D))
        yh = (y // 32).astype(np.float16)
        yl = (y % 32).astype(np.float16)
        yhr, ylr = yh[roll], yl[roll]
        zfeat = np.stack([np.ones(N, np.float16), -2 * yhr, yhr * yhr,
                          np.ones(N, np.float16), -2 * ylr, ylr * ylr]).astype(np.float16)
        yhl, yll = yh[rows], yl[rows]
        zrow = np.stack([yhl * yhl, yhl, np.ones(NL, np.float16),
                         yll * yll, yll, np.ones(NL, np.float16)]).astype(np.float16)
        mask8 = np.zeros((128, 1), dtype=np.float32)
        for p in range(64):
            c, s = divmod(p, NCORES)
            if (c + s) % NCORES == r:
                mask8[p, 0] = 1.0
        in_maps.append({
            "qT": qT, "kr": kr, "qr": qr, "ybc": ybc, "yrow": yrow,
            "eyem": eyem, "mask8": mask8, "zfeat": zfeat, "zrow": zrow,
        })
    return in_maps


_CACHE = {}


def _get_nc(cfg_key):
    if cfg_key not in _CACHE:
        cfg = Cfg()
        _CACHE[cfg_key] = (cfg, build_bass(cfg))
    return _CACHE[cfg_key]


def kernel(q, k, y, trace=False):
    cfg, nc = _get_nc("full")
    in_maps = make_inputs(q, k, y, cfg)
    res = run_bass_kernel_spmd(nc, in_maps, core_ids=list(range(NCORES)),
                               trace=trace)
    total = np.sum([res.results[r]["out"][0, 0] for r in range(NCORES)],
                   dtype=np.float64)
    out = np.asarray(total / cfg.N, dtype=np.float32)
    if trace:
        kernel.last_results = res
    return out


# revision 38
# speedup vs baseline: 1.4770x; 1.0062x over previous
"""Trainium2 Bass kernel for the supervised-contrastive loss (nn_KCL_69784628626020).

Strategy (8 NeuronCores, SPMD), v3:
  - Shard anchors (rows of q, k, y) across cores: 1024 rows/core.
  - Each core computes its [1024, 8192] slab of EW = exp((q_loc @ q_full^T
    + lw_j)/TAU) = w_j * exp(S/TAU) on the tensor engine (bf16 operands,
    fp32 PSUM, a rank-1 matmul folds the per-column lw_j = -TAU*ln(count_j)).
  - COLUMN ROTATION: core r's column order is rolled by r*1024 so that the
    self-similarity (diagonal) entry of local row-block b always lands in the
    static window [b*128, (b+1)*128).  A tiny eye-masked reduce extracts the
    exact stored w*E_ii per row.
  - Per row i (sums include the diagonal; it cancels exactly):
        A_i  = sum_j w_j E_ij       (FREE: activation accum_out during exp)
        SCw_i = sum_{y_j==y_i} w_j E_ij   (DVE fused masked reduce / quarter)
        den_i = log(A_i - SCw_i)          (diagonal cancels exactly)
        num_i = log(kpos_i + count_i*(SCw_i - wE_ii))
        loss_i = (den_i - num_i) / (count_i - 1 + K)
  - Class counts are computed on device (DVE label-equality reduces); the
    rotated lw row is assembled with a mask-shifted ReduceScatter (core r
    contributes lw_r into shard c slot (r-c)%8, so the scatter hands each
    core its rotated vector) -- pure data-driven, SPMD-safe.
  - kpos_i = sum_k exp(q_i . k_ik / TAU): GpSimd multiplies, ACT accumulates
    + exponentiates.
  - Final mean: ones-matmul partition reduction; host adds the 8 partials.
"""

import numpy as np
from contextlib import ExitStack

import concourse.bass as bass
import concourse.bacc as bacc
import concourse.tile as tile
from concourse import mybir
from concourse.bass_utils import run_bass_kernel_spmd
import ml_dtypes

F32 = mybir.dt.float32
F16 = mybir.dt.float16
BF16 = mybir.dt.bfloat16
AL = mybir.AluOpType
AF = mybir.ActivationFunctionType

TAU = 0.07
NCORES = 8


class Cfg:
    def __init__(self, N=8192, D=512, KP=8, NQ=4):
        self.N, self.D, self.KP, self.NQ = N, D, KP, NQ
        self.NL = N // NCORES      # rows per core
        self.NB = self.NL // 128   # 128-row blocks per core
        self.KC = D // 128         # contraction chunks
        self.QW = N // NQ          # column quarter width
        self.NCH = max(1, self.QW // 512)
        self.CW = self.QW // self.NCH   # matmul chunk width
        assert self.NL % 128 == 0 and self.QW % self.NCH == 0
        assert self.CW <= 512


def build_bass(cfg: Cfg, debug_out=False):
    N, D, KP, NQ = cfg.N, cfg.D, cfg.KP, cfg.NQ
    NL, NB, KC, QW, NCH, CW = cfg.NL, cfg.NB, cfg.KC, cfg.QW, cfg.NCH, cfg.CW
    NQT = NB * NQ              # total quarters

    nc = bacc.Bacc("TRN2", target_bir_lowering=False, debug=False,
                   num_devices=NCORES)

    # ---- kernel I/O -------------------------------------------------------
    qT_d = nc.dram_tensor("qT", [KC, 128, N], BF16, kind="ExternalInput")
    kr_d = nc.dram_tensor("kr", [NB, 128, KP * D], BF16, kind="ExternalInput")
    qr_d = nc.dram_tensor("qr", [NB, 128, D], BF16, kind="ExternalInput")
    ybc_d = nc.dram_tensor("ybc", [128, N], F16, kind="ExternalInput")
    yrow_d = nc.dram_tensor("yrow", [128, NB], F32, kind="ExternalInput")
    eyem_d = nc.dram_tensor("eyem", [128, 128], F16, kind="ExternalInput")
    mask8_d = nc.dram_tensor("mask8", [128, 1], F32, kind="ExternalInput")
    zfeat_d = nc.dram_tensor("zfeat", [6, N], F16, kind="ExternalInput")
    zrow_d = nc.dram_tensor("zrow", [6, NL], F16, kind="ExternalInput")
    out_d = nc.dram_tensor("out", [1, 1], F32, kind="ExternalOutput")
    if debug_out:
        dlw_d = nc.dram_tensor("dlw", [1, N], BF16, kind="ExternalOutput")
        dfin_d = nc.dram_tensor("dfin", [128, 8 * NB], F32, kind="ExternalOutput")

    with tile.TileContext(nc) as tc, ExitStack() as ctx:
        const = ctx.enter_context(tc.tile_pool(name="const", bufs=1))
        ew_pool = ctx.enter_context(tc.tile_pool(name="ew", bufs=6))
        psum_pool = ctx.enter_context(tc.tile_pool(name="ps", bufs=2, space="PSUM"))
        k_pool = ctx.enter_context(tc.tile_pool(name="kp", bufs=1))
        q_pool = ctx.enter_context(tc.tile_pool(name="qp", bufs=2))
        ks_pool = ctx.enter_context(tc.tile_pool(name="ks", bufs=1))
        dram = ctx.enter_context(tc.tile_pool(name="dram", bufs=1, space="DRAM"))

        # ---- resident constants ------------------------------------------
        qts = [const.tile([128, N], BF16, tag=f"qts{c}", name=f"qts{c}")
               for c in range(KC)]
        ybc = const.tile([128, N], F16, tag="ybc")
        yrow = const.tile([128, NB], F32, tag="yrow")
        nc.sync.dma_start(yrow[:, :], yrow_d[:, :])
        eyem = const.tile([128, 128], F16, tag="eyem")
        nc.sync.dma_start(eyem[:, :], eyem_d[:, :])
        mask8 = const.tile([128, 1], F32, tag="mask8")
        nc.sync.dma_start(mask8[:, :], mask8_d[:, :])
        zrow = const.tile([6, NL], F16, tag="zrow")
        nc.sync.dma_start(zrow[:, :], zrow_d[:, :])
        zfeat = const.tile([6, N], F16, tag="zfeat")
        nc.sync.dma_start(zfeat[:, :], zfeat_d[:, :])
        nc.sync.dma_start(ybc[:, 0:QW], ybc_d[:, 0:QW])
        nc.sync.dma_start(ybc[:, QW:N], ybc_d[:, QW:N])
        for c in range(KC):
            nc.sync.dma_start(qts[c][:, 0:QW], qT_d[c, :, 0:QW])
        # k-path inputs early so GpSimd products (and ACT reduces) start soon
        kts = []
        qts_k = []
        for b in range(NB):
            kt = k_pool.tile([128, KP * D], BF16, tag="kt", name=f"kt{b}")
            nc.sync.dma_start(kt[:, :], kr_d[b, :, :])
            qt = q_pool.tile([128, D], BF16, tag="qt", name=f"qt{b}")
            nc.sync.dma_start(qt[:, :], qr_d[b, :, :])
            kts.append(kt)
            qts_k.append(qt)
        for qq in range(1, NQ):
            for c in range(KC):
                nc.sync.dma_start(qts[c][:, qq * QW:(qq + 1) * QW],
                                  qT_d[c, :, qq * QW:(qq + 1) * QW])

        ones_col = const.tile([128, 1], F32, tag="ones_col")
        nc.vector.memset(ones_col[:, :], 1.0)
        ones_k1 = const.tile([1, 128], BF16, tag="ones_k1")
        nc.vector.memset(ones_k1[:, :], 1.0)

        # accumulator slots
        scs = const.tile([128, NQT], F32, tag="scs")
        aw = const.tile([128, NQT], F32, tag="aw")
        eh = const.tile([128, NB], F32, tag="eh")
        kss = const.tile([128, NB * KP], F32, tag="kss")
        kpos = const.tile([128, NB], F32, tag="kpos")
        cloc = const.tile([128, NB], F32, tag="cloc")
        clocB = const.tile([128, NB], F32, tag="clocB")

        # ---- k-path products on GpSimd (first half only, so the lw-phase
        # collective trigger is not queued behind all 64 products) ---------
        kscrs = []

        def emit_kprods(b0, b1):
            for b in range(b0, b1):
                kscr = ks_pool.tile([128, KP * D], BF16, tag="kscr",
                                    name=f"kscr{b}")
                for kk in range(KP):
                    nc.gpsimd.tensor_tensor(
                        kscr[:, kk * D:(kk + 1) * D],
                        kts[b][:, kk * D:(kk + 1) * D], qts_k[b][:, :],
                        op=AL.mult)
                kscrs.append(kscr)

        NBH = 1
        emit_kprods(0, NBH)

        # ---- class counts via PE z-trick + ACT exp-accum -----------------
        # z_ij = (hi_i-hi_j)^2 + (lo_i-lo_j)^2 is an exact small integer in
        # fp16/fp32 (labels split base-32); count_i = sum_j exp(-30*z_ij).
        # Runs on the otherwise-idle PE/ACT while lwrow is being assembled.
        zcs = const.tile([128, NQT], F32, tag="zcs")
        zdump = const.tile([128, QW], F32, tag="zdump")
        zdumpv = const.tile([128, QW], F16, tag="zdumpv")
        for b in range(NB):
            for qq in range(NQ):
                m = b * NQ + qq
                zps = psum_pool.tile([128, QW], F32, tag="ps")
                for ch in range(NCH):
                    nc.tensor.matmul(
                        zps[:, ch * CW:(ch + 1) * CW],
                        zrow[:, b * 128:(b + 1) * 128],
                        zfeat[:, qq * QW + ch * CW: qq * QW + (ch + 1) * CW],
                        start=True, stop=True)
                if m % 2 == 0:
                    nc.scalar.activation(zdump[:, :], zps[:, :], AF.Exp,
                                         scale=float(-30.0),
                                         accum_out=zcs[:, m:m + 1])
                else:
                    nc.vector.tensor_scalar(
                        zdumpv[:, :], zps[:, :], 0.0, None,
                        op0=AL.is_equal, op1=AL.add,
                        accum_out=zcs[:, m:m + 1])
        nc.vector.tensor_reduce(
            cloc[:, :], zcs[:, :].rearrange("p (b q) -> p b q", b=NB, q=NQ),
            mybir.AxisListType.X, AL.add)

        # ---- lw row: lw = -TAU*ln(count); rotated share via RS -----------
        lnc = const.tile([128, NB], F32, tag="lnc")
        nc.scalar.activation(lnc[:, :], cloc[:, :], AF.Ln)
        lwloc = const.tile([128, NB], F32, tag="lwloc")
        nc.vector.tensor_scalar_mul(lwloc[:, :], lnc[:, :], -TAU)
        lw16 = const.tile([128, NB], BF16, tag="lw16")
        nc.vector.tensor_copy(lw16[:, :], lwloc[:, :])
        lwpad = const.tile([128, 32], BF16, tag="lwpad")
        nc.vector.memset(lwpad[:, :], 0.0)
        nc.vector.tensor_copy(lwpad[:, 0:NB], lw16[:, :])
        lwT = const.tile([128, 32], BF16, tag="lwT")
        nc.vector.transpose(lwT[:, :], lwpad[:, :])
        # lwpart[0, i*128+a*32+j] = lw(local row i*128+a*32+j) = lwT[a*32+i, j]
        lwpart = dram.tile([1, NL], BF16)
        dst3 = lwpart[:, :].rearrange("o (i a j) -> a i (o j)", i=NB, a=4, j=32)
        for a in range(4):
            nc.gpsimd.dma_start(dst3[a], lwT[a * 32:a * 32 + NB, 0:32])
        # ReduceScatter rotation: core r contributes lw_r to shard c slot
        # (r-c)%8; the scatter hands core c exactly its rotated lw vector.
        bufc = const.tile([128, NL], BF16, tag="bufc")
        nc.gpsimd.dma_start(bufc[0:64, :],
                            lwpart[0:1, :].partition_broadcast(64))
        nc.vector.tensor_scalar_mul(bufc[0:64, :], bufc[0:64, :],
                                    mask8[0:64, 0:1])
        bufc_d = dram.tile([1, 64 * NL], BF16)
        nc.gpsimd.dma_start(
            bufc_d[:, :].rearrange("o (p g) -> p (o g)", p=64, g=NL),
            bufc[0:64, :])
        lwrs_d = dram.tile([1, NCORES * NL], BF16)
        nc.gpsimd.collective_compute(
            "ReduceScatter", AL.add,
            ins=[bufc_d[:, :].opt()],
            outs=[lwrs_d[:, :].opt()],
            replica_groups=[list(range(NCORES))],
        )
        lwrow = const.tile([1, N], BF16, tag="lwrow")
        nc.gpsimd.dma_start(lwrow[0:1, :], lwrs_d[0:1, :])
        emit_kprods(NBH, NB)

        # ---- main loop ----------------------------------------------------
        # PE: quarters of the weighted score slab (q-chunks + lw rank-1).
        # ACT: exp with accum (A row-sum free) + k-path reduces interleaved.
        # DVE: SCw masked reduce + diag extract.
        sc_scr = const.tile([128, QW], F32, tag="sc_scr")
        eh_scr = const.tile([128, 128], F32, tag="eh_scr")
        kdump = const.tile([128, D], BF16, tag="kdump")

        kred_jobs = [(b, kk) for b in range(NB) for kk in range(KP)]
        kred_pos = 0

        def emit_kred(n):
            nonlocal kred_pos
            for _ in range(n):
                if kred_pos >= len(kred_jobs):
                    return
                b, kk = kred_jobs[kred_pos]
                nc.scalar.activation(
                    kdump[:, :],
                    kscrs[b][:, kk * D:(kk + 1) * D],
                    AF.Copy, accum_out=kss[:, b * KP + kk: b * KP + kk + 1])
                kred_pos += 1

        emit_kred(len(kred_jobs))

        for b in range(NB):
            q0t = None
            for qq in range(NQ):
                m = b * NQ + qq
                ewt = ew_pool.tile([128, QW], F32, tag="ew", name=f"ew{m}")
                if qq == 0:
                    q0t = ewt
                ps = psum_pool.tile([128, QW], F32, tag="ps")
                for c in range(KC):
                    for ch in range(NCH):
                        o = ps[:, ch * CW:(ch + 1) * CW]
                        nc.tensor.matmul(
                            o,
                            qts[c][:, b * 128:(b + 1) * 128],
                            qts[c][:, qq * QW + ch * CW: qq * QW + (ch + 1) * CW],
                            start=(c == 0), stop=False)
                for ch in range(NCH):
                    o = ps[:, ch * CW:(ch + 1) * CW]
                    nc.tensor.matmul(
                        o, ones_k1[0:1, :],
                        lwrow[0:1, qq * QW + ch * CW: qq * QW + (ch + 1) * CW],
                        start=False, stop=True)
                ewq = ewt[:, :]
                nc.scalar.activation(ewq, ps[:, :], AF.Exp,
                                     scale=float(1.0 / TAU),
                                     accum_out=aw[:, m:m + 1])
                # SCw: same-class weighted row-sum (incl diag) on DVE
                nc.vector.scalar_tensor_tensor(
                    sc_scr[:, :], ybc[:, qq * QW:(qq + 1) * QW],
                    yrow[:, b:b + 1], ewq,
                    op0=AL.is_equal, op1=AL.mult,
                    accum_out=scs[:, m:m + 1])
            # exact diagonal extraction from the static rotated window
            # (local block b's diagonal lies in quarter 0 of its slab)
            nc.vector.scalar_tensor_tensor(
                eh_scr[:, :], eyem[:, :], 1.0, q0t[:, b * 128:(b + 1) * 128],
                op0=AL.mult, op1=AL.mult,
                accum_out=eh[:, b:b + 1])

        emit_kred(len(kred_jobs))

        # ---- k-path exps --------------------------------------------------
        ksse = const.tile([128, NB * KP], F32, tag="ksse")
        for b in range(NB):
            nc.scalar.activation(
                ksse[:, b * KP:(b + 1) * KP],
                kss[:, b * KP:(b + 1) * KP],
                AF.Exp, scale=float(1.0 / TAU),
                accum_out=kpos[:, b:b + 1])

        # ---- finalize (wide [128, NB] ops) --------------------------------
        SC = const.tile([128, NB], F32, tag="SC")
        A = const.tile([128, NB], F32, tag="A")
        nc.vector.tensor_reduce(
            SC[:, :], scs[:, :].rearrange("p (b q) -> p b q", b=NB, q=NQ),
            mybir.AxisListType.X, AL.add)
        nc.vector.tensor_reduce(
            A[:, :], aw[:, :].rearrange("p (b q) -> p b q", b=NB, q=NQ),
            mybir.AxisListType.X, AL.add)
        numin = const.tile([128, NB], F32, tag="numin")
        tmp = const.tile([128, NB], F32, tag="tmp")
        densub = const.tile([128, NB], F32, tag="densub")
        # num_in = kpos + cloc * (SCw - eh)
        nc.vector.tensor_sub(tmp[:, :], SC[:, :], eh[:, :])
        nc.vector.tensor_mul(tmp[:, :], tmp[:, :], cloc[:, :])
        nc.vector.tensor_add(numin[:, :], tmp[:, :], kpos[:, :])
        # den_in = A - SCw  (diagonal and same-class weights cancel exactly)
        nc.vector.tensor_sub(densub[:, :], A[:, :], SC[:, :])
        den_l = const.tile([128, NB], F32, tag="den_l")
        num_l = const.tile([128, NB], F32, tag="num_l")
        nc.scalar.activation(den_l[:, :], densub[:, :], AF.Ln)
        nc.scalar.activation(num_l[:, :], numin[:, :], AF.Ln)
        # loss rows: (den_l - num_l) / (cloc - 1 + KP)
        ctil = const.tile([128, NB], F32, tag="ctil")
        dinv = const.tile([128, NB], F32, tag="dinv")
        nc.vector.tensor_scalar_add(ctil[:, :], cloc[:, :], float(KP - 1))
        nc.vector.reciprocal(dinv[:, :], ctil[:, :])
        diff = const.tile([128, NB], F32, tag="diff")
        lossrow = const.tile([128, NB], F32, tag="lossrow")
        nc.vector.tensor_sub(diff[:, :], den_l[:, :], num_l[:, :])
        nc.vector.tensor_mul(lossrow[:, :], diff[:, :], dinv[:, :])

        # ---- reduce to a single partial ----------------------------------
        lsum = const.tile([128, 1], F32, tag="lsum")
        nc.vector.tensor_reduce(lsum[:, :], lossrow[:, :],
                                mybir.AxisListType.X, AL.add)
        psf = psum_pool.tile([128, QW], F32, tag="ps")
        nc.tensor.matmul(psf[0:1, 0:1], lsum[:, :],
                         ones_col[:, :], start=True, stop=True)
        outsb = const.tile([1, 1], F32, tag="outsb")
        nc.scalar.copy(outsb[0:1, 0:1], psf[0:1, 0:1])
        nc.sync.dma_start(out_d[:, :], outsb[0:1, 0:1])

        if debug_out:
            nc.sync.dma_start(dlw_d[:, :], lwrow[0:1, :])
            dfin = const.tile([128, 8 * NB], F32, tag="dfin")
            for i, t in enumerate([SC, A, eh, kpos, cloc, densub, numin,
                                   lossrow]):
                nc.vector.tensor_copy(dfin[:, i * NB:(i + 1) * NB], t[:, :])
            nc.sync.dma_start(dfin_d[:, :], dfin[:, :])

    nc.compile()
    return nc


# ---------------------------------------------------------------------------
# host-side marshalling
# ---------------------------------------------------------------------------

def make_inputs(q, k, y, cfg: Cfg):
    """Build the per-core input maps (pure layout/replication marshalling)."""
    N, D, KP = cfg.N, cfg.D, cfg.KP
    NL, NB, KC = cfg.NL, cfg.NB, cfg.KC
    q = np.asarray(q, dtype=np.float32)
    k = np.asarray(k, dtype=np.float32)
    y = np.asarray(y)

    qbf = q.astype(ml_dtypes.bfloat16)
    qTf = np.ascontiguousarray(qbf.T)           # [D, N]
    eyem = np.eye(128, dtype=np.float16)

    in_maps = []
    for r in range(NCORES):
        rows = slice(r * NL, (r + 1) * NL)
        roll = (np.arange(N) + r * NL) % N
        qT = np.ascontiguousarray(qTf[:, roll]).reshape(KC, 128, N)
        ybc = np.broadcast_to(y[roll].astype(np.float16)[None, :], (128, N)).copy()
        yrow = np.ascontiguousarray(
            y[rows].astype(np.float32).reshape(NB, 128).T)
        kr = np.ascontiguousarray(
            k[rows].reshape(NB, 128, KP * D)).astype(ml_dtypes.bfloat16)
        qr = np.ascontiguousarray(qbf[rows].reshape(NB, 128, D))
        yh = (y // 32).astype(np.float16)
        yl = (y % 32).astype(np.float16)
        yhr, ylr = yh[roll], yl[roll]
        zfeat = np.stack([np.ones(N, np.float16), -2 * yhr, yhr * yhr,
                          np.ones(N, np.float16), -2 * ylr, ylr * ylr]).astype(np.float16)
        yhl, yll = yh[rows], yl[rows]
        zrow = np.stack([yhl * yhl, yhl, np.ones(NL, np.float16),
                         yll * yll, yll, np.ones(NL, np.float16)]).astype(np.float16)
        mask8 = np.zeros((128, 1), dtype=np.float32)
        for p in range(64):
            c, s = divmod(p, NCORES)
            if (c + s) % NCORES == r:
                mask8[p, 0] = 1.0
        in_maps.append({
            "qT": qT, "kr": kr, "qr": qr, "ybc": ybc, "yrow": yrow,
            "eyem": eyem, "mask8": mask8, "zfeat": zfeat, "zrow": zrow,
        })
    return in_maps


_CACHE = {}


def _get_nc(cfg_key):
    if cfg_key not in _CACHE:
        cfg = Cfg()
        _CACHE[cfg_key] = (cfg, build_bass(cfg))
    return _CACHE[cfg_key]


def kernel(q, k, y, trace=False):
    cfg, nc = _get_nc("full")
    in_maps = make_inputs(q, k, y, cfg)
    res = run_bass_kernel_spmd(nc, in_maps, core_ids=list(range(NCORES)),
                               trace=trace)
    total = np.sum([res.results[r]["out"][0, 0] for r in range(NCORES)],
                   dtype=np.float64)
    out = np.asarray(total / cfg.N, dtype=np.float32)
    if trace:
        kernel.last_results = res
    return out
